# revision 16
# baseline (speedup 1.0000x reference)
"""Trainium2 Bass kernel: single-head causal self-attention.

Reference computation (per batch b):
    Q = x @ Wq ; K = x @ Wk ; V = x @ Wv          (x: [S, D])
    S_sc = Q @ K^T / sqrt(D), causal masked
    out  = softmax(S_sc) @ V

Sharding: 8 cores, 4 batches -> core c handles batch b = c//2 and query
half h = c%2 (1024 query rows), with full K/V for that batch computed
on-core (redundantly for the pair). Uniform SPMD program; per-core
behavior comes only from input data (xqT slice + global-q-index vector
used to build the causal mask on device).

Layout strategy (all fp32):
  - Host passes x[b]^T so the contraction dim (d_in) lands on partitions.
  - K^T [d, S] stays resident in SBUF; V [S, d] is staged to DRAM during
    the projection phase and streamed back per q-strip.
  - Scores are computed TRANSPOSED: S^T[k, q] = sum_d K^T[d,k] * Q^T[d,q],
    so softmax's k-reduction lands on the partition dim; the sum is done
    with an extra N=1 matmul against a ones vector (riding the same
    stationary P^T tile as the P@V matmuls), avoiding any P transposes.
  - No max-subtraction in softmax: scores ~ N(0,1), exp is safe in fp32.
  - Causal mask built on device: mask[k,q] = (q_global >= k_global),
    multiplied into exp(S^T) (multiplicative 0/1 mask after exp).
"""

import sys

try:
    import concourse.bass as bass  # noqa: F401
except ImportError:
    sys.path.insert(0, "/opt/trn_rl_repo")

import numpy as np

import concourse.bass as bass
import concourse.tile as tile
from concourse import bacc, mybir
from concourse.bass_utils import run_bass_kernel_spmd

B, S, D = 4, 2048, 1024
NQ = 1024  # query rows per core
NK = 2048  # keys per core
P = 128
DT = D // P  # 8 d tiles
KT = NK // P  # 16 k tiles
W = 256  # q-strip width
NSTRIP = NQ // W  # 4 strips
F32 = mybir.dt.float32
SCALE = 1.0 / np.sqrt(np.float32(D))  # 0.03125

_NC_CACHE = {}


def build_nc(mm_dt=F32):
    nc = bacc.Bacc(None, target_bir_lowering=False)
    xkvT = nc.dram_tensor("xkvT", [D, NK], mm_dt, kind="ExternalInput")
    xqT = nc.dram_tensor("xqT", [D, NQ], mm_dt, kind="ExternalInput")
    qg = nc.dram_tensor("qg", [NQ], F32, kind="ExternalInput")
    wq_d = nc.dram_tensor("Wq", [D, D], mm_dt, kind="ExternalInput")
    wk_d = nc.dram_tensor("Wk", [D, D], mm_dt, kind="ExternalInput")
    wv_d = nc.dram_tensor("Wv", [D, D], mm_dt, kind="ExternalInput")
    out_d = nc.dram_tensor("out", [NQ, D], F32, kind="ExternalOutput")
    vdram = nc.dram_tensor("vscratch", [NK, D], mm_dt)
    qtdram = nc.dram_tensor("qtscratch", [D, NQ], mm_dt)

    with tile.TileContext(nc) as tc:
        with (
            tc.tile_pool(name="persist", bufs=1) as persist,
            tc.tile_pool(name="misc", bufs=1) as misc,
        ):
            # Persistent K^T [d, NK] (8 partition-tiles)
            kT = persist.tile([P, DT, NK], mm_dt, tag="kT")

            # Small constants: ones columns (2 wide: fp32r matmuls need an
            # even moving dim), k-index vectors for the mask
            ones_f = misc.tile([P, 2], F32, tag="ones_f")
            nc.vector.memset(ones_f, 1.0)
            ones = misc.tile([P, 2], mm_dt, tag="ones")
            nc.vector.tensor_copy(ones, ones_f)
            pvec_i = misc.tile([P, 1], mybir.dt.int32, tag="pvec_i")
            nc.gpsimd.iota(pvec_i, pattern=[[0, 1]], base=0, channel_multiplier=1)
            pvec = misc.tile([P, 1], F32, tag="pvec")
            nc.vector.tensor_copy(pvec, pvec_i)
            kvecf = misc.tile([P, KT], F32, tag="kvecf")
            for kt in range(KT):
                nc.vector.tensor_scalar_add(kvecf[:, kt : kt + 1], pvec, float(kt * P))

            # ------------- Phase 0: Q^T -> DRAM (so phase 2 needs no
            # weight loads and starts without an SBUF WAR stall) -------------
            qtdram_t = qtdram.rearrange("(a p) q -> p a q", p=P)
            xqT_t = xqT.rearrange("(a p) s -> p a s", p=P)
            with (
                tc.tile_pool(name="wqp", bufs=1) as wqp,
                tc.tile_pool(name="qproj", bufs=2) as qproj,
                tc.tile_pool(name="ps0", bufs=4, space="PSUM") as ps0,
            ):
                wq = wqp.tile([P, DT, D], mm_dt, tag="wq")
                wq_t = wq_d.rearrange("(a p) o -> p a o", p=P)
                for di in range(DT):
                    nc.scalar.dma_start(wq[:, di, :], wq_t[:, di, :])
                for qs in range(NSTRIP):
                    q0 = qs * W
                    xqs = qproj.tile([P, DT, W], mm_dt, tag="xqs")
                    nc.scalar.dma_start(xqs, xqT_t[:, :, q0 : q0 + W])
                    qstage = qproj.tile([P, DT, W], mm_dt, tag="qstage")
                    for do in range(DT):
                        ps = ps0.tile([P, W], F32, tag="ps0")
                        for di in range(DT):
                            nc.tensor.matmul(
                                ps,
                                wq[:, di, do * P : (do + 1) * P],
                                xqs[:, di, :],
                                start=(di == 0),
                                stop=(di == DT - 1),
                            )
                        nc.vector.tensor_copy(qstage[:, do, :], ps)
                    nc.sync.dma_start(qtdram_t[:, :, q0 : q0 + W], qstage)

            # ------------- Phase 1: K^T (SBUF) and V (-> DRAM) -------------
            with (
                tc.tile_pool(name="wkv", bufs=1) as wkvp,
                tc.tile_pool(name="xin", bufs=2) as xinp,
                tc.tile_pool(name="vstage", bufs=2) as vsp,
                tc.tile_pool(name="ps1", bufs=4, space="PSUM") as ps1,
            ):
                wk = wkvp.tile([P, DT, D], mm_dt, tag="wk")
                wv = wkvp.tile([P, DT, D], mm_dt, tag="wv")
                wk_t = wk_d.rearrange("(a p) o -> p a o", p=P)
                wv_t = wv_d.rearrange("(a p) o -> p a o", p=P)
                for di in range(DT):
                    nc.scalar.dma_start(wk[:, di, :], wk_t[:, di, :])
                    nc.scalar.dma_start(wv[:, di, :], wv_t[:, di, :])
                xkvT_t = xkvT.rearrange("(a p) s -> p a s", p=P)

                for qr in range(4):  # quarters of the key sequence
                    s0 = qr * 512
                    xin = xinp.tile([P, DT, 512], mm_dt, tag="xin")
                    nc.sync.dma_start(xin, xkvT_t[:, :, s0 : s0 + 512])
                    # K^T tiles: out[d_out, s] accumulated over d_in
                    for do in range(DT):
                        ps = ps1.tile([P, 512], F32, tag="ps1")
                        for di in range(DT):
                            nc.tensor.matmul(
                                ps,
                                wk[:, di, do * P : (do + 1) * P],
                                xin[:, di, :],
                                start=(di == 0),
                                stop=(di == DT - 1),
                            )
                        nc.vector.tensor_copy(kT[:, do, s0 : s0 + 512], ps)
                    # V tiles: out[s, d_out] accumulated over d_in -> DRAM
                    for st in range(4):
                        gst = qr * 4 + st
                        vstage = vsp.tile([P, D], mm_dt, tag="vstage")
                        for dh in range(2):
                            ps = ps1.tile([P, 512], F32, tag="ps1")
                            for di in range(DT):
                                nc.tensor.matmul(
                                    ps,
                                    xin[:, di, st * P : (st + 1) * P],
                                    wv[:, di, dh * 512 : (dh + 1) * 512],
                                    start=(di == 0),
                                    stop=(di == DT - 1),
                                )
                            nc.vector.tensor_copy(vstage[:, dh * 512 : (dh + 1) * 512], ps)
                        nc.sync.dma_start(vdram[gst * P : (gst + 1) * P, :], vstage)

            # ---------------- Phase 2: per-q-strip attention ----------------
            with (
                tc.tile_pool(name="strip", bufs=1) as strip,
                tc.tile_pool(name="vs2", bufs=3) as vs2,
                tc.tile_pool(name="sm", bufs=4) as sm,
                tc.tile_pool(name="outp", bufs=2) as outp,
                tc.tile_pool(name="ps2", bufs=2, space="PSUM") as ps2p,
                tc.tile_pool(name="psc", bufs=2, space="PSUM") as pscp,
                tc.tile_pool(name="psl", bufs=2, space="PSUM") as pslp,
            ):
                for qs in range(NSTRIP):
                    q0 = qs * W
                    qT = strip.tile([P, DT, W], mm_dt, tag="qT", bufs=2)
                    nc.scalar.dma_start(qT, qtdram_t[:, :, q0 : q0 + W])

                    # broadcast global q indices for this strip to all partitions
                    qgrid = sm.tile([P, W], F32, tag="qgrid")
                    qg_sl = qg[q0 : q0 + W]
                    nc.gpsimd.dma_start(
                        qgrid,
                        bass.AP(
                            tensor=qg_sl.tensor,
                            offset=qg_sl.offset,
                            ap=[[0, P]] + list(qg_sl.ap),
                        ),
                    )

                    # S^T strip -> exp -> mask -> P^T strip.
                    # Causal interleave: this strip holds global q-tiles
                    # 2j+h for j in {2qs, 2qs+1}, so k-tiles >= ext_kt are
                    # fully masked and skipped at compile time.
                    ext_kt = 4 * (qs + 1)
                    pT = strip.tile([P, KT, W], mm_dt, tag="pT")
                    for kt in range(ext_kt):
                        ps = ps2p.tile([P, W], F32, tag="ps2")
                        for di in range(DT):
                            nc.tensor.matmul(
                                ps,
                                kT[:, di, kt * P : (kt + 1) * P],
                                qT[:, di, :],
                                start=(di == 0),
                                stop=(di == DT - 1),
                            )
                        et = sm.tile([P, W], F32, tag="et")
                        nc.scalar.activation(
                            et, ps, mybir.ActivationFunctionType.Exp, scale=float(SCALE)
                        )
                        mt = sm.tile([P, W], F32, tag="mt")
                        nc.vector.tensor_scalar(
                            mt,
                            qgrid,
                            kvecf[:, kt : kt + 1],
                            None,
                            op0=mybir.AluOpType.is_ge,
                        )
                        nc.vector.tensor_mul(pT[:, kt, :], et, mt)

                    # context = P^T.T @ V (V streamed from DRAM, kt-outer),
                    # row-sums l via ones column riding the same stationary P^T
                    ncq = W // P
                    cps = [
                        pscp.tile([P, D], F32, tag="psc", name=f"cps{i}")
                        for i in range(ncq)
                    ]
                    lps = [
                        pslp.tile([P, 2], F32, tag="psl", name=f"lps{i}")
                        for i in range(ncq)
                    ]
                    for kt in range(ext_kt):
                        vt_t = vs2.tile([P, D], mm_dt, tag="vstrip")
                        nc.sync.dma_start(vt_t, vdram[kt * P : (kt + 1) * P, :])
                        for qt in range(ncq):
                            ej = 2 * (qs * ncq + qt) + 2  # this position's extent
                            if kt >= ej:
                                continue
                            lhs = pT[:, kt, qt * P : (qt + 1) * P]
                            nc.tensor.matmul(
                                cps[qt][:, 0:512],
                                lhs,
                                vt_t[:, 0:512],
                                start=(kt == 0),
                                stop=(kt == ej - 1),
                            )
                            nc.tensor.matmul(
                                cps[qt][:, 512:1024],
                                lhs,
                                vt_t[:, 512:1024],
                                start=(kt == 0),
                                stop=(kt == ej - 1),
                            )
                            nc.tensor.matmul(
                                lps[qt],
                                lhs,
                                ones,
                                start=(kt == 0),
                                stop=(kt == ej - 1),
                            )
                    for qt in range(ncq):
                        qrow = q0 + qt * P
                        rt = sm.tile([P, 1], F32, tag="rt")
                        nc.vector.reciprocal(rt, lps[qt][:, 0:1])
                        ot = outp.tile([P, D], F32, tag="ot")
                        nc.vector.tensor_scalar_mul(ot, cps[qt], rt)
                        nc.sync.dma_start(out_d[qrow : qrow + P, :], ot)
    nc.compile()
    return nc


def _get_nc(key="f32"):
    if key not in _NC_CACHE:
        _NC_CACHE[key] = build_nc(F32 if key == "f32" else mybir.dt.float32r)
    return _NC_CACHE[key]


def _qsel(h):
    """Query rows for core-half h: global q-tiles h, 2+h, ..., 14+h.

    Position j's tile 2j+h needs only k < (2j+h+1)*128, letting the kernel
    skip fully-masked k-tiles at compile time with a core-uniform program."""
    tiles = np.arange(8) * 2 + h
    return (tiles[:, None] * P + np.arange(P)[None, :]).reshape(-1)


def make_in_maps(x, Wq, Wk, Wv):
    x = np.asarray(x, dtype=np.float32)
    Wq = np.ascontiguousarray(np.asarray(Wq, dtype=np.float32))
    Wk = np.ascontiguousarray(np.asarray(Wk, dtype=np.float32))
    Wv = np.ascontiguousarray(np.asarray(Wv, dtype=np.float32))
    in_maps = []
    for c in range(8):
        b, h = c // 2, c % 2
        qsel = _qsel(h)
        in_maps.append(
            {
                "xkvT": np.ascontiguousarray(x[b].T),
                "xqT": np.ascontiguousarray(x[b][qsel].T),
                "qg": qsel.astype(np.float32),
                "Wq": Wq,
                "Wk": Wk,
                "Wv": Wv,
            }
        )
    return in_maps


def kernel(x, Wq, Wk, Wv, _trace=False, _nc_key="f32"):
    nc = _get_nc(_nc_key)
    in_maps = make_in_maps(x, Wq, Wk, Wv)
    res = run_bass_kernel_spmd(nc, in_maps, core_ids=list(range(8)), trace=_trace)
    out = np.empty((B, S, D), dtype=np.float32)
    for c in range(8):
        b, h = c // 2, c % 2
        out[b, _qsel(h), :] = res.results[c]["out"]
    if _trace:
        kernel.last_results = res
    return out


# revision 17
# speedup vs baseline: 1.1304x; 1.1304x over previous
"""Trainium2 Bass kernel: single-head causal self-attention.

Reference computation (per batch b):
    Q = x @ Wq ; K = x @ Wk ; V = x @ Wv          (x: [S, D])
    S_sc = Q @ K^T / sqrt(D), causal masked
    out  = softmax(S_sc) @ V

Sharding: 8 cores, 4 batches -> core c handles batch b = c//2 and query
half h = c%2 (1024 query rows), with full K/V for that batch computed
on-core (redundantly for the pair). Uniform SPMD program; per-core
behavior comes only from input data (xqT slice + global-q-index vector
used to build the causal mask on device).

Layout strategy (all fp32):
  - Host passes x[b]^T so the contraction dim (d_in) lands on partitions.
  - K^T [d, S] stays resident in SBUF; V [S, d] is staged to DRAM during
    the projection phase and streamed back per q-strip.
  - Scores are computed TRANSPOSED: S^T[k, q] = sum_d K^T[d,k] * Q^T[d,q],
    so softmax's k-reduction lands on the partition dim; the sum is done
    with an extra N=1 matmul against a ones vector (riding the same
    stationary P^T tile as the P@V matmuls), avoiding any P transposes.
  - No max-subtraction in softmax: scores ~ N(0,1), exp is safe in fp32.
  - Causal mask built on device: mask[k,q] = (q_global >= k_global),
    multiplied into exp(S^T) (multiplicative 0/1 mask after exp).
"""

import sys

try:
    import concourse.bass as bass  # noqa: F401
except ImportError:
    sys.path.insert(0, "/opt/trn_rl_repo")

import numpy as np

import concourse.bass as bass
import concourse.tile as tile
from concourse import bacc, mybir
from concourse.bass_utils import run_bass_kernel_spmd

B, S, D = 4, 2048, 1024
NQ = 1024  # query rows per core
NK = 2048  # keys per core
P = 128
DT = D // P  # 8 d tiles
KT = NK // P  # 16 k tiles
W = 256  # q-strip width
NSTRIP = NQ // W  # 4 strips
F32 = mybir.dt.float32
SCALE = 1.0 / np.sqrt(np.float32(D))  # 0.03125

_NC_CACHE = {}


def build_nc(mm_dt=F32):
    nc = bacc.Bacc(None, target_bir_lowering=False)
    xkvT = nc.dram_tensor("xkvT", [D, NK], mm_dt, kind="ExternalInput")
    xqT = nc.dram_tensor("xqT", [D, NQ], mm_dt, kind="ExternalInput")
    qg = nc.dram_tensor("qg", [NQ], F32, kind="ExternalInput")
    wq_d = nc.dram_tensor("Wq", [D, D], mm_dt, kind="ExternalInput")
    wk_d = nc.dram_tensor("Wk", [D, D], mm_dt, kind="ExternalInput")
    wv_d = nc.dram_tensor("Wv", [D, D], mm_dt, kind="ExternalInput")
    out_d = nc.dram_tensor("out", [NQ, D], F32, kind="ExternalOutput")
    vdram = nc.dram_tensor("vscratch", [NK, D], mm_dt)

    with tile.TileContext(nc) as tc:
        with (
            tc.tile_pool(name="persist", bufs=1) as persist,
            tc.tile_pool(name="misc", bufs=1) as misc,
        ):
            # Persistent K^T [d, NK] (8 partition-tiles)
            kT = persist.tile([P, DT, NK], mm_dt, tag="kT")

            # Small constants: ones columns (2 wide: fp32r matmuls need an
            # even moving dim), k-index vectors for the mask
            ones_f = misc.tile([P, 2], F32, tag="ones_f")
            nc.vector.memset(ones_f, 1.0)
            ones = misc.tile([P, 2], mm_dt, tag="ones")
            nc.vector.tensor_copy(ones, ones_f)
            pvec_i = misc.tile([P, 1], mybir.dt.int32, tag="pvec_i")
            nc.gpsimd.iota(pvec_i, pattern=[[0, 1]], base=0, channel_multiplier=1)
            pvec = misc.tile([P, 1], F32, tag="pvec")
            nc.vector.tensor_copy(pvec, pvec_i)
            kvecf = misc.tile([P, KT], F32, tag="kvecf")
            for kt in range(KT):
                nc.vector.tensor_scalar_add(kvecf[:, kt : kt + 1], pvec, float(kt * P))

            # ------------- Phase 1: K^T (SBUF) and V (-> DRAM) -------------
            with (
                tc.tile_pool(name="wkv", bufs=1) as wkvp,
                tc.tile_pool(name="xin", bufs=2) as xinp,
                tc.tile_pool(name="vstage", bufs=2) as vsp,
                tc.tile_pool(name="ps1", bufs=4, space="PSUM") as ps1,
            ):
                wk = wkvp.tile([P, DT, D], mm_dt, tag="wk")
                wv = wkvp.tile([P, DT, D], mm_dt, tag="wv")
                wk_t = wk_d.rearrange("(a p) o -> p a o", p=P)
                wv_t = wv_d.rearrange("(a p) o -> p a o", p=P)
                for di in range(DT):
                    nc.scalar.dma_start(wk[:, di, :], wk_t[:, di, :])
                    nc.scalar.dma_start(wv[:, di, :], wv_t[:, di, :])
                xkvT_t = xkvT.rearrange("(a p) s -> p a s", p=P)

                for qr in range(4):  # quarters of the key sequence
                    s0 = qr * 512
                    xin = xinp.tile([P, DT, 512], mm_dt, tag="xin")
                    nc.sync.dma_start(xin, xkvT_t[:, :, s0 : s0 + 512])
                    # K^T tiles: out[d_out, s] accumulated over d_in
                    for do in range(DT):
                        ps = ps1.tile([P, 512], F32, tag="ps1")
                        for di in range(DT):
                            nc.tensor.matmul(
                                ps,
                                wk[:, di, do * P : (do + 1) * P],
                                xin[:, di, :],
                                start=(di == 0),
                                stop=(di == DT - 1),
                            )
                        nc.vector.tensor_copy(kT[:, do, s0 : s0 + 512], ps)
                    # V tiles: out[s, d_out] accumulated over d_in -> DRAM
                    for st in range(4):
                        gst = qr * 4 + st
                        vstage = vsp.tile([P, D], mm_dt, tag="vstage")
                        for dh in range(2):
                            ps = ps1.tile([P, 512], F32, tag="ps1")
                            for di in range(DT):
                                nc.tensor.matmul(
                                    ps,
                                    xin[:, di, st * P : (st + 1) * P],
                                    wv[:, di, dh * 512 : (dh + 1) * 512],
                                    start=(di == 0),
                                    stop=(di == DT - 1),
                                )
                            nc.vector.tensor_copy(vstage[:, dh * 512 : (dh + 1) * 512], ps)
                        nc.sync.dma_start(vdram[gst * P : (gst + 1) * P, :], vstage)

            # ---------------- Phase 2: per-q-strip attention ----------------
            with (
                tc.tile_pool(name="wqp", bufs=1) as wqp,
                tc.tile_pool(name="strip", bufs=1) as strip,
                tc.tile_pool(name="vs2", bufs=3) as vs2,
                tc.tile_pool(name="sm", bufs=4) as sm,
                tc.tile_pool(name="outp", bufs=2) as outp,
                tc.tile_pool(name="ps2", bufs=2, space="PSUM") as ps2p,
                tc.tile_pool(name="psc", bufs=2, space="PSUM") as pscp,
                tc.tile_pool(name="psl", bufs=2, space="PSUM") as pslp,
            ):
                wq = wqp.tile([P, DT, D], mm_dt, tag="wq")
                wq_t = wq_d.rearrange("(a p) o -> p a o", p=P)
                for di in range(DT):
                    nc.scalar.dma_start(wq[:, di, :], wq_t[:, di, :])
                xqT_t = xqT.rearrange("(a p) s -> p a s", p=P)

                for qs in range(NSTRIP):
                    q0 = qs * W
                    qx = strip.tile([P, DT, W], mm_dt, tag="qx", bufs=2)
                    nc.scalar.dma_start(qx, xqT_t[:, :, q0 : q0 + W])
                    # Q^T strip [d, W]
                    qT = strip.tile([P, DT, W], mm_dt, tag="qT")
                    for do in range(DT):
                        ps = ps2p.tile([P, W], F32, tag="ps2")
                        for di in range(DT):
                            nc.tensor.matmul(
                                ps,
                                wq[:, di, do * P : (do + 1) * P],
                                qx[:, di, :],
                                start=(di == 0),
                                stop=(di == DT - 1),
                            )
                        nc.vector.tensor_copy(qT[:, do, :], ps)

                    # broadcast global q indices for this strip to all partitions
                    qgrid = sm.tile([P, W], F32, tag="qgrid")
                    qg_sl = qg[q0 : q0 + W]
                    nc.gpsimd.dma_start(
                        qgrid,
                        bass.AP(
                            tensor=qg_sl.tensor,
                            offset=qg_sl.offset,
                            ap=[[0, P]] + list(qg_sl.ap),
                        ),
                    )

                    # S^T strip -> exp -> mask -> P^T strip.
                    # Causal interleave: this strip holds global q-tiles
                    # 2j+h for j in {2qs, 2qs+1}, so k-tiles >= ext_kt are
                    # fully masked and skipped at compile time.
                    ext_kt = 4 * (qs + 1)
                    pT = strip.tile([P, KT, W], mm_dt, tag="pT")
                    for kt in range(ext_kt):
                        ps = ps2p.tile([P, W], F32, tag="ps2")
                        for di in range(DT):
                            nc.tensor.matmul(
                                ps,
                                kT[:, di, kt * P : (kt + 1) * P],
                                qT[:, di, :],
                                start=(di == 0),
                                stop=(di == DT - 1),
                            )
                        et = sm.tile([P, W], F32, tag="et")
                        nc.scalar.activation(
                            et, ps, mybir.ActivationFunctionType.Exp, scale=float(SCALE)
                        )
                        mt = sm.tile([P, W], F32, tag="mt")
                        nc.vector.tensor_scalar(
                            mt,
                            qgrid,
                            kvecf[:, kt : kt + 1],
                            None,
                            op0=mybir.AluOpType.is_ge,
                        )
                        nc.vector.tensor_mul(pT[:, kt, :], et, mt)

                    # context = P^T.T @ V (V streamed from DRAM, kt-outer),
                    # row-sums l via ones column riding the same stationary P^T
                    ncq = W // P
                    cps = [
                        pscp.tile([P, D], F32, tag="psc", name=f"cps{i}")
                        for i in range(ncq)
                    ]
                    lps = [
                        pslp.tile([P, 2], F32, tag="psl", name=f"lps{i}")
                        for i in range(ncq)
                    ]
                    for kt in range(ext_kt):
                        vt_t = vs2.tile([P, D], mm_dt, tag="vstrip")
                        nc.sync.dma_start(vt_t, vdram[kt * P : (kt + 1) * P, :])
                        for qt in range(ncq):
                            ej = 2 * (qs * ncq + qt) + 2  # this position's extent
                            if kt >= ej:
                                continue
                            lhs = pT[:, kt, qt * P : (qt + 1) * P]
                            nc.tensor.matmul(
                                cps[qt][:, 0:512],
                                lhs,
                                vt_t[:, 0:512],
                                start=(kt == 0),
                                stop=(kt == ej - 1),
                            )
                            nc.tensor.matmul(
                                cps[qt][:, 512:1024],
                                lhs,
                                vt_t[:, 512:1024],
                                start=(kt == 0),
                                stop=(kt == ej - 1),
                            )
                            nc.tensor.matmul(
                                lps[qt],
                                lhs,
                                ones,
                                start=(kt == 0),
                                stop=(kt == ej - 1),
                            )
                    for qt in range(ncq):
                        qrow = q0 + qt * P
                        rt = sm.tile([P, 1], F32, tag="rt")
                        nc.vector.reciprocal(rt, lps[qt][:, 0:1])
                        ot = outp.tile([P, D], F32, tag="ot")
                        nc.vector.tensor_scalar_mul(ot, cps[qt], rt)
                        nc.sync.dma_start(out_d[qrow : qrow + P, :], ot)
    nc.compile()
    return nc


def _get_nc(key="f32"):
    if key not in _NC_CACHE:
        _NC_CACHE[key] = build_nc(F32 if key == "f32" else mybir.dt.float32r)
    return _NC_CACHE[key]


def _qsel(h):
    """Query rows for core-half h: global q-tiles h, 2+h, ..., 14+h.

    Position j's tile 2j+h needs only k < (2j+h+1)*128, letting the kernel
    skip fully-masked k-tiles at compile time with a core-uniform program."""
    tiles = np.arange(8) * 2 + h
    return (tiles[:, None] * P + np.arange(P)[None, :]).reshape(-1)


def make_in_maps(x, Wq, Wk, Wv):
    x = np.asarray(x, dtype=np.float32)
    Wq = np.ascontiguousarray(np.asarray(Wq, dtype=np.float32))
    Wk = np.ascontiguousarray(np.asarray(Wk, dtype=np.float32))
    Wv = np.ascontiguousarray(np.asarray(Wv, dtype=np.float32))
    in_maps = []
    for c in range(8):
        b, h = c // 2, c % 2
        qsel = _qsel(h)
        in_maps.append(
            {
                "xkvT": np.ascontiguousarray(x[b].T),
                "xqT": np.ascontiguousarray(x[b][qsel].T),
                "qg": qsel.astype(np.float32),
                "Wq": Wq,
                "Wk": Wk,
                "Wv": Wv,
            }
        )
    return in_maps


def kernel(x, Wq, Wk, Wv, _trace=False, _nc_key="f32r"):
    nc = _get_nc(_nc_key)
    in_maps = make_in_maps(x, Wq, Wk, Wv)
    res = run_bass_kernel_spmd(nc, in_maps, core_ids=list(range(8)), trace=_trace)
    out = np.empty((B, S, D), dtype=np.float32)
    for c in range(8):
        b, h = c // 2, c % 2
        out[b, _qsel(h), :] = res.results[c]["out"]
    if _trace:
        kernel.last_results = res
    return out


# revision 18
# speedup vs baseline: 1.1836x; 1.0471x over previous
"""Trainium2 Bass kernel: single-head causal self-attention.

Reference computation (per batch b):
    Q = x @ Wq ; K = x @ Wk ; V = x @ Wv          (x: [S, D])
    S_sc = Q @ K^T / sqrt(D), causal masked
    out  = softmax(S_sc) @ V

Sharding: 8 cores, 4 batches -> core c handles batch b = c//2 and query
half h = c%2 (1024 query rows), with full K/V for that batch computed
on-core (redundantly for the pair). Uniform SPMD program; per-core
behavior comes only from input data (xqT slice + global-q-index vector
used to build the causal mask on device).

Layout strategy (all fp32):
  - Host passes x[b]^T so the contraction dim (d_in) lands on partitions.
  - K^T [d, S] stays resident in SBUF; V [S, d] is staged to DRAM during
    the projection phase and streamed back per q-strip.
  - Scores are computed TRANSPOSED: S^T[k, q] = sum_d K^T[d,k] * Q^T[d,q],
    so softmax's k-reduction lands on the partition dim; the sum is done
    with an extra N=1 matmul against a ones vector (riding the same
    stationary P^T tile as the P@V matmuls), avoiding any P transposes.
  - No max-subtraction in softmax: scores ~ N(0,1), exp is safe in fp32.
  - Causal mask built on device: mask[k,q] = (q_global >= k_global),
    multiplied into exp(S^T) (multiplicative 0/1 mask after exp).
"""

import sys

try:
    import concourse.bass as bass  # noqa: F401
except ImportError:
    sys.path.insert(0, "/opt/trn_rl_repo")

import numpy as np

import concourse.bass as bass
import concourse.tile as tile
from concourse import bacc, mybir
from concourse.bass_utils import run_bass_kernel_spmd

B, S, D = 4, 2048, 1024
NQ = 1024  # query rows per core
NK = 2048  # keys per core
P = 128
DT = D // P  # 8 d tiles
KT = NK // P  # 16 k tiles
W = 256  # q-strip width
NSTRIP = NQ // W  # 4 strips
F32 = mybir.dt.float32
SCALE = 1.0 / np.sqrt(np.float32(D))  # 0.03125

_NC_CACHE = {}


def build_nc(mm_dt=F32):
    nc = bacc.Bacc(None, target_bir_lowering=False)
    xkvT = nc.dram_tensor("xkvT", [D, NK], mm_dt, kind="ExternalInput")
    xqT = nc.dram_tensor("xqT", [D, NQ], mm_dt, kind="ExternalInput")
    qg = nc.dram_tensor("qg", [NQ], F32, kind="ExternalInput")
    wq_d = nc.dram_tensor("Wq", [D, D], mm_dt, kind="ExternalInput")
    wk_d = nc.dram_tensor("Wk", [D, D], mm_dt, kind="ExternalInput")
    wv_d = nc.dram_tensor("Wv", [D, D], mm_dt, kind="ExternalInput")
    out_d = nc.dram_tensor("out", [NQ, D], F32, kind="ExternalOutput")
    vdram = nc.dram_tensor("vscratch", [NK, D], mm_dt)

    with tile.TileContext(nc) as tc:
        with (
            tc.tile_pool(name="persist", bufs=1) as persist,
            tc.tile_pool(name="misc", bufs=1) as misc,
        ):
            # Persistent K^T [d, NK] (8 partition-tiles)
            kT = persist.tile([P, DT, NK], mm_dt, tag="kT")

            # Small constants: ones columns (2 wide: fp32r matmuls need an
            # even moving dim), k-index vectors for the mask
            ones_f = misc.tile([P, 2], F32, tag="ones_f")
            nc.vector.memset(ones_f, 1.0)
            ones = misc.tile([P, 2], mm_dt, tag="ones")
            nc.vector.tensor_copy(ones, ones_f)
            pvec_i = misc.tile([P, 1], mybir.dt.int32, tag="pvec_i")
            nc.gpsimd.iota(pvec_i, pattern=[[0, 1]], base=0, channel_multiplier=1)
            pvec = misc.tile([P, 1], F32, tag="pvec")
            nc.vector.tensor_copy(pvec, pvec_i)
            kvecf = misc.tile([P, KT], F32, tag="kvecf")
            for kt in range(KT):
                nc.vector.tensor_scalar_add(kvecf[:, kt : kt + 1], pvec, float(kt * P))

            # ------------- Phase 1: K^T (SBUF) and V (-> DRAM) -------------
            with (
                tc.tile_pool(name="wkv", bufs=1) as wkvp,
                tc.tile_pool(name="xin", bufs=2) as xinp,
                tc.tile_pool(name="vstage", bufs=2) as vsp,
                tc.tile_pool(name="ps1", bufs=4, space="PSUM") as ps1,
            ):
                wk = wkvp.tile([P, DT, D], mm_dt, tag="wk")
                wv = wkvp.tile([P, DT, D], mm_dt, tag="wv")
                wk_t = wk_d.rearrange("(a p) o -> p a o", p=P)
                wv_t = wv_d.rearrange("(a p) o -> p a o", p=P)
                # all of wk first: the first K^T group accumulates over all
                # 8 d_in tiles, so wk's arrival gates PE start; wv is not
                # needed until the V section
                for di in range(DT):
                    nc.scalar.dma_start(wk[:, di, :], wk_t[:, di, :])
                for di in range(DT):
                    nc.scalar.dma_start(wv[:, di, :], wv_t[:, di, :])
                xkvT_t = xkvT.rearrange("(a p) s -> p a s", p=P)

                for qr in range(4):  # quarters of the key sequence
                    s0 = qr * 512
                    xin = xinp.tile([P, DT, 512], mm_dt, tag="xin")
                    nc.sync.dma_start(xin, xkvT_t[:, :, s0 : s0 + 512])
                    # K^T tiles: out[d_out, s] accumulated over d_in
                    for do in range(DT):
                        ps = ps1.tile([P, 512], F32, tag="ps1")
                        for di in range(DT):
                            nc.tensor.matmul(
                                ps,
                                wk[:, di, do * P : (do + 1) * P],
                                xin[:, di, :],
                                start=(di == 0),
                                stop=(di == DT - 1),
                            )
                        nc.vector.tensor_copy(kT[:, do, s0 : s0 + 512], ps)
                    # V tiles: out[s, d_out] accumulated over d_in -> DRAM
                    for st in range(4):
                        gst = qr * 4 + st
                        vstage = vsp.tile([P, D], mm_dt, tag="vstage")
                        for dh in range(2):
                            ps = ps1.tile([P, 512], F32, tag="ps1")
                            for di in range(DT):
                                nc.tensor.matmul(
                                    ps,
                                    xin[:, di, st * P : (st + 1) * P],
                                    wv[:, di, dh * 512 : (dh + 1) * 512],
                                    start=(di == 0),
                                    stop=(di == DT - 1),
                                )
                            nc.vector.tensor_copy(vstage[:, dh * 512 : (dh + 1) * 512], ps)
                        nc.sync.dma_start(vdram[gst * P : (gst + 1) * P, :], vstage)

            # ---------------- Phase 2: per-q-strip attention ----------------
            with (
                tc.tile_pool(name="wqp", bufs=1) as wqp,
                tc.tile_pool(name="strip", bufs=1) as strip,
                tc.tile_pool(name="vs2", bufs=4) as vs2,
                tc.tile_pool(name="sm", bufs=4) as sm,
                tc.tile_pool(name="outp", bufs=2) as outp,
                tc.tile_pool(name="ps2", bufs=2, space="PSUM") as ps2p,
                tc.tile_pool(name="psc", bufs=2, space="PSUM") as pscp,
                tc.tile_pool(name="psl", bufs=2, space="PSUM") as pslp,
            ):
                wq = wqp.tile([P, DT, D], mm_dt, tag="wq")
                wq_t = wq_d.rearrange("(a p) o -> p a o", p=P)
                for di in range(DT):
                    nc.scalar.dma_start(wq[:, di, :], wq_t[:, di, :])
                xqT_t = xqT.rearrange("(a p) s -> p a s", p=P)

                for qs in range(NSTRIP):
                    q0 = qs * W
                    qx = strip.tile([P, DT, W], mm_dt, tag="qx", bufs=2)
                    nc.scalar.dma_start(qx, xqT_t[:, :, q0 : q0 + W])
                    # Q^T strip [d, W]
                    qT = strip.tile([P, DT, W], mm_dt, tag="qT")
                    for do in range(DT):
                        ps = ps2p.tile([P, W], F32, tag="ps2")
                        for di in range(DT):
                            nc.tensor.matmul(
                                ps,
                                wq[:, di, do * P : (do + 1) * P],
                                qx[:, di, :],
                                start=(di == 0),
                                stop=(di == DT - 1),
                            )
                        nc.vector.tensor_copy(qT[:, do, :], ps)

                    # broadcast global q indices for this strip to all partitions
                    qgrid = sm.tile([P, W], F32, tag="qgrid")
                    qg_sl = qg[q0 : q0 + W]
                    nc.gpsimd.dma_start(
                        qgrid,
                        bass.AP(
                            tensor=qg_sl.tensor,
                            offset=qg_sl.offset,
                            ap=[[0, P]] + list(qg_sl.ap),
                        ),
                    )

                    # S^T strip -> exp -> mask -> P^T strip.
                    # Causal interleave: this strip holds global q-tiles
                    # 2j+h for j in {2qs, 2qs+1}, so k-tiles >= ext_kt are
                    # fully masked and skipped at compile time.
                    ext_kt = 4 * (qs + 1)
                    pT = strip.tile([P, KT, W], mm_dt, tag="pT")
                    for kt in range(ext_kt):
                        ps = ps2p.tile([P, W], F32, tag="ps2")
                        for di in range(DT):
                            nc.tensor.matmul(
                                ps,
                                kT[:, di, kt * P : (kt + 1) * P],
                                qT[:, di, :],
                                start=(di == 0),
                                stop=(di == DT - 1),
                            )
                        et = sm.tile([P, W], F32, tag="et")
                        nc.scalar.activation(
                            et, ps, mybir.ActivationFunctionType.Exp, scale=float(SCALE)
                        )
                        mt = sm.tile([P, W], F32, tag="mt")
                        nc.vector.tensor_scalar(
                            mt,
                            qgrid,
                            kvecf[:, kt : kt + 1],
                            None,
                            op0=mybir.AluOpType.is_ge,
                        )
                        nc.vector.tensor_mul(pT[:, kt, :], et, mt)

                    # context = P^T.T @ V (V streamed from DRAM, kt-outer),
                    # row-sums l via ones column riding the same stationary P^T
                    ncq = W // P
                    cps = [
                        pscp.tile([P, D], F32, tag="psc", name=f"cps{i}")
                        for i in range(ncq)
                    ]
                    lps = [
                        pslp.tile([P, 2], F32, tag="psl", name=f"lps{i}")
                        for i in range(ncq)
                    ]
                    for kt in range(ext_kt):
                        vt_t = vs2.tile([P, D], mm_dt, tag="vstrip")
                        nc.sync.dma_start(vt_t, vdram[kt * P : (kt + 1) * P, :])
                        for qt in range(ncq):
                            ej = 2 * (qs * ncq + qt) + 2  # this position's extent
                            if kt >= ej:
                                continue
                            lhs = pT[:, kt, qt * P : (qt + 1) * P]
                            nc.tensor.matmul(
                                cps[qt][:, 0:512],
                                lhs,
                                vt_t[:, 0:512],
                                start=(kt == 0),
                                stop=(kt == ej - 1),
                            )
                            nc.tensor.matmul(
                                cps[qt][:, 512:1024],
                                lhs,
                                vt_t[:, 512:1024],
                                start=(kt == 0),
                                stop=(kt == ej - 1),
                            )
                            nc.tensor.matmul(
                                lps[qt],
                                lhs,
                                ones,
                                start=(kt == 0),
                                stop=(kt == ej - 1),
                            )
                    for qt in range(ncq):
                        qrow = q0 + qt * P
                        rt = sm.tile([P, 1], F32, tag="rt")
                        nc.vector.reciprocal(rt, lps[qt][:, 0:1])
                        ot = outp.tile([P, D], F32, tag="ot")
                        nc.vector.tensor_scalar_mul(ot, cps[qt], rt)
                        nc.sync.dma_start(out_d[qrow : qrow + P, :], ot)
    nc.compile()
    return nc


def _get_nc(key="f32"):
    if key not in _NC_CACHE:
        _NC_CACHE[key] = build_nc(F32 if key == "f32" else mybir.dt.float32r)
    return _NC_CACHE[key]


def _qsel(h):
    """Query rows for core-half h: global q-tiles h, 2+h, ..., 14+h.

    Position j's tile 2j+h needs only k < (2j+h+1)*128, letting the kernel
    skip fully-masked k-tiles at compile time with a core-uniform program."""
    tiles = np.arange(8) * 2 + h
    return (tiles[:, None] * P + np.arange(P)[None, :]).reshape(-1)


def make_in_maps(x, Wq, Wk, Wv):
    x = np.asarray(x, dtype=np.float32)
    Wq = np.ascontiguousarray(np.asarray(Wq, dtype=np.float32))
    Wk = np.ascontiguousarray(np.asarray(Wk, dtype=np.float32))
    Wv = np.ascontiguousarray(np.asarray(Wv, dtype=np.float32))
    in_maps = []
    for c in range(8):
        b, h = c // 2, c % 2
        qsel = _qsel(h)
        in_maps.append(
            {
                "xkvT": np.ascontiguousarray(x[b].T),
                "xqT": np.ascontiguousarray(x[b][qsel].T),
                "qg": qsel.astype(np.float32),
                "Wq": Wq,
                "Wk": Wk,
                "Wv": Wv,
            }
        )
    return in_maps


def kernel(x, Wq, Wk, Wv, _trace=False, _nc_key="f32r"):
    nc = _get_nc(_nc_key)
    in_maps = make_in_maps(x, Wq, Wk, Wv)
    res = run_bass_kernel_spmd(nc, in_maps, core_ids=list(range(8)), trace=_trace)
    out = np.empty((B, S, D), dtype=np.float32)
    for c in range(8):
        b, h = c // 2, c % 2
        out[b, _qsel(h), :] = res.results[c]["out"]
    if _trace:
        kernel.last_results = res
    return out
